# revision 41
# baseline (speedup 1.0000x reference)
"""Trainium2 Bass kernel for nn_NodeModel (GNN message passing).

Reference computation:
    h   = relu(concat(x[row], edge_attr) @ W1 + b1) @ W2 + b2     # edge MLP
    agg = scatter_mean(h, col, N)                                  # per-dest mean
    out = relu(concat(x, agg) @ W3 + b3) @ W4 + b4                 # node MLP

Distribution strategy (8 cores, no collectives needed):
  - Sort edges by destination node; split destination nodes into 8
    block-aligned, edge-balanced shards.  Each core owns one node shard and
    ALL edges targeting it, so per-node sums are complete locally.
  - x is replicated; each core gathers x[row] rows with indirect DMA.

Key design points (vs the fp32r baseline at ~2.0ms; now ~0.80ms):
  - All matmuls in bf16 (1 cycle/row on the PE, like fp32r, but half the
    SBUF/DMA footprint).  fp8+DoubleRow was measured and rejected: any
    single e4m3-quantized operand already costs 1.5-3e-2 relative error
    vs the 2e-2 tolerance (errors compose in quadrature).
  - scatter_mean commutes with the (linear) W2 matmul: only
    g = relu(cat @ W1)/cnt is computed per edge; W2 is applied per node.
    Further, g only feeds z @ W3, so W2 @ W3[FN:] is folded into ONE
    host-precomputed weight W23: the entire per-node W2 stage vanishes.
    Device FLOPs drop ~2.4x vs the reference dataflow.
  - MM1 emits edge-major (W1 moving, cat^T stationary), so the per-edge
    tensor g needs no exit transposes.  relu and the per-edge 1/cnt
    scale (a per-partition scale in edge-major layout) fold into the
    PSUM drain.  Relies on b1 == b2 == 0 (asserted; true for this model).
  - The scatter emits agg^T directly (aggT[f,n] += g[e,fslice]^T @ S[e,n]
    with one-hot S built by is_equal): no agg transposes.  PSUM start
    bits zero a whole 2KB bank, so the four f-regions sharing a bank get
    one start=True on first touch only.
  - MM4 streams W4 with h3^T stationary, emitting output node-major: no
    output transposes.  Phase 2 contains zero PE transposes (HAM-safe).
  - g staged to DRAM in bf16 and gathered per 128-node block by SW-DGE
    (schedule width KB = max chunks per block, exact).
"""

import math
import sys
from contextlib import ExitStack

sys.path.insert(0, "/opt/trn_rl_repo")

import ml_dtypes
import numpy as np

import concourse.bass as bass
import concourse.tile as tile
from concourse import bacc, mybir
from concourse.bass_utils import run_bass_kernel_spmd

NCORES = 8
P = 128
FN = 512    # node feature dim
FE = 128    # edge feature dim
HID = 1280  # edge-MLP hidden/output dim
INS = FN + FE           # 640  edge-MLP input
IN2 = FN + HID          # 1792 node-MLP input
F32 = mybir.dt.float32
BF16 = mybir.dt.bfloat16
F8 = mybir.dt.float8e4
I32 = mybir.dt.int32
RELU = mybir.ActivationFunctionType.Relu
COPY = mybir.ActivationFunctionType.Copy
DR = mybir.MatmulPerfMode.DoubleRow
MULT = mybir.AluOpType.mult

# ---- precision config (validated against the reference on CPU) ----
EDGE_F8 = False    # x-gather, edge_attr, W1, h1, W2
H2O_F8 = False     # h2 staging + scatter (False -> bf16)
X3_F8 = False      # x rows in node MLP + W3 x-part
AGG_F8 = False     # agg (aggT) + W3 agg-part
H3_F8 = False      # h3 + W4

NP_F8 = ml_dtypes.float8_e4m3
NP_BF16 = ml_dtypes.bfloat16


def _dt(flag):
    return F8 if flag else BF16


def _npdt(flag):
    return NP_F8 if flag else NP_BF16


def _pow2scale(w):
    m = float(np.abs(w).max())
    if m == 0.0:
        return 1.0
    return 2.0 ** math.floor(math.log2(224.0 / m)) / 2


_prog_cache = {}


def _build(EC, NB, KB, NX):
    """Build the SPMD program for one core.

    EC: edge chunks (128 edges each) per core, multiple of 4.
    NB: node blocks (128 nodes each) per core, multiple of 4.
    KB: max edge chunks per node block (scatter schedule width), even.
    NX: number of rows of the replicated x (gather source).
    """
    EP = EC * P
    NBP = NB * P
    SC = EC // 4   # superchunks of 512 edges
    NSB = NB // 4  # superblocks of 512 nodes

    DT_E = _dt(EDGE_F8)
    DT_H = _dt(H2O_F8)
    DT_X3 = _dt(X3_F8)
    DT_A = _dt(AGG_F8)
    DT_H3 = _dt(H3_F8)

    nc = bacc.Bacc("TRN2", target_bir_lowering=False, debug=False,
                   num_devices=NCORES)

    x_d = nc.dram_tensor("x", [NX, FN], DT_E, kind="ExternalInput")
    rows_d = nc.dram_tensor("rows", [P, EC], I32, kind="ExternalInput")
    eaT_d = nc.dram_tensor("eaT", [FE, EP], DT_E, kind="ExternalInput")
    W1_d = nc.dram_tensor("W1", [INS, HID], DT_E, kind="ExternalInput")
    W3x_d = nc.dram_tensor("W3x", [FN, INS], DT_X3, kind="ExternalInput")
    # W3a holds the host-precomputed W2 @ W3[FN:]: the scatter-mean output
    # g feeds W3 through W2, both linear, so the two weights fuse.
    W3a_d = nc.dram_tensor("W3a", [HID, INS], DT_A, kind="ExternalInput")
    W4_d = nc.dram_tensor("W4", [INS, FN], DT_H3, kind="ExternalInput")
    b1_d = nc.dram_tensor("b1", [P, HID // P], F32, kind="ExternalInput")
    b3_d = nc.dram_tensor("b3", [P, INS // P], F32, kind="ExternalInput")
    scE_d = nc.dram_tensor("scE", [P, EC], F32, kind="ExternalInput")
    gid_d = nc.dram_tensor("gid", [P, NB * KB], I32, kind="ExternalInput")
    colb_d = nc.dram_tensor("colb", [P, NB * KB], F32, kind="ExternalInput")
    xsT_d = nc.dram_tensor("xsT", [FN, NBP], DT_X3, kind="ExternalInput")
    iota_d = nc.dram_tensor("iota", [P, P], F32, kind="ExternalInput")
    idE_d = nc.dram_tensor("idE", [P, P], DT_E, kind="ExternalInput")
    out_d = nc.dram_tensor("out", [NBP, FN], F32, kind="ExternalOutput")
    h2_d = nc.dram_tensor("h2buf", [EP, HID], DT_H)  # internal staging

    # weight scales folded into drains (host passes pre-scaled weights)
    inv_s1_d = nc.dram_tensor("inv_s1", [P, 1], F32, kind="ExternalInput")
    inv_s3_d = nc.dram_tensor("inv_s3", [P, 1], F32, kind="ExternalInput")
    inv_s4_d = nc.dram_tensor("inv_s4", [P, 1], F32, kind="ExternalInput")

    with tile.TileContext(nc) as tc, ExitStack() as ctx:
        cpool = ctx.enter_context(tc.tile_pool(name="const", bufs=1))

        idEt = cpool.tile([P, P], DT_E)
        nc.sync.dma_start(idEt[:], idE_d.ap()[:])
        iotat = cpool.tile([P, P], F32)
        nc.sync.dma_start(iotat[:], iota_d.ap()[:])
        b1t = cpool.tile([P, HID // P], F32)
        nc.sync.dma_start(b1t[:], b1_d.ap()[:])
        b3t = cpool.tile([P, INS // P], F32)
        nc.sync.dma_start(b3t[:], b3_d.ap()[:])
        scEt = cpool.tile([P, EC], F32)
        nc.sync.dma_start(scEt[:], scE_d.ap()[:])
        rowst = cpool.tile([P, EC], I32)
        nc.sync.dma_start(rowst[:], rows_d.ap()[:])
        gidt = cpool.tile([P, NB * KB], I32)
        nc.sync.dma_start(gidt[:], gid_d.ap()[:])
        colbt = cpool.tile([P, NB * KB], F32)
        nc.sync.dma_start(colbt[:], colb_d.ap()[:])
        is1t = cpool.tile([P, 1], F32)
        nc.sync.dma_start(is1t[:], inv_s1_d.ap()[:])
        is3t = cpool.tile([P, 1], F32)
        nc.sync.dma_start(is3t[:], inv_s3_d.ap()[:])
        is4t = cpool.tile([P, 1], F32)
        nc.sync.dma_start(is4t[:], inv_s4_d.ap()[:])

        # Phase-2 weights + first x-shard tile: loaded up-front so their
        # DMAs don't queue behind all of phase 1's h2-staging writes.
        # ---------------- Phase E: edge half-MLP ----------------
        # Stages g_e = relu(cat(x[row], ea) @ W1) / cnt[col(e)] per edge.
        # scatter_mean commutes with the (linear) W2 matmul + b2 (b2==0),
        # so W2 is applied per *node* in phase 2: 2.56x less W2 work.
        with ExitStack() as ectx:
            wpool = ectx.enter_context(tc.tile_pool(name="wE", bufs=1))
            W1t = wpool.tile([P, 5, HID], DT_E)
            W1r = W1_d.ap().rearrange("(ko ki) m -> ki ko m", ki=P)
            for k in range(5):
                nc.sync.dma_start(W1t[:, k, :], W1r[:, k, :])

            ptp = ectx.enter_context(
                tc.tile_pool(name="ptp", bufs=2, space="PSUM"))
            xgp = ectx.enter_context(tc.tile_pool(name="xg", bufs=2))
            xgTp = ectx.enter_context(tc.tile_pool(name="xgT", bufs=2))
            eap = ectx.enter_context(tc.tile_pool(name="ea", bufs=2))
            h2op = ectx.enter_context(tc.tile_pool(name="h2o", bufs=4))
            mmp = ectx.enter_context(
                tc.tile_pool(name="mmE", bufs=4, space="PSUM"))

            def issue_gather(sc):
                xgt = xgp.tile([P, 4, FN], DT_E)
                for k in range(4):
                    nc.gpsimd.indirect_dma_start(
                        out=xgt[:, k, :], out_offset=None, in_=x_d.ap()[:],
                        in_offset=bass.IndirectOffsetOnAxis(
                            ap=rowst[:, sc * 4 + k:sc * 4 + k + 1], axis=0))
                eat = eap.tile([P, 512], DT_E)
                nc.sync.dma_start(
                    eat[:], eaT_d.ap()[:, sc * 512:(sc + 1) * 512])
                return xgt, eat

            def entry_T(xgt, xgTt, f, k):
                pt = ptp.tile([P, P], DT_E)
                nc.tensor.transpose(
                    pt[:], xgt[:, k, f * P:(f + 1) * P], idEt[:])
                nc.vector.tensor_copy(xgTt[:, f, k * P:(k + 1) * P], pt[:])

            # prologue: superchunk 0 input + its entry transposes
            xg_cur, ea_cur = issue_gather(0)
            xgT_cur = xgTp.tile([P, 4, 512], DT_E)
            for f in range(4):
                for k in range(4):
                    entry_T(xg_cur, xgT_cur, f, k)

            for sc in range(SC):
                if sc + 1 < SC:
                    xg_next, ea_next = issue_gather(sc + 1)
                    xgT_next = xgTp.tile([P, 4, 512], DT_E)
                    tq = [(f, k) for f in range(4) for k in range(4)]
                else:
                    xg_next = ea_next = xgT_next = None
                    tq = []

                def drip_T(n):
                    for _ in range(n):
                        if tq:
                            f, k = tq.pop(0)
                            entry_T(xg_next, xgT_next, f, k)

                # MM1 edge-major: per 128-edge chunk, W1 moving,
                # cat^T slices stationary.  Drain: relu then scale by
                # (1/cnt)/s1 per edge (b1==0; relu commutes with the
                # positive scale).
                for ec in range(4):
                    c = sc * 4 + ec
                    h2ot = h2op.tile([P, HID], DT_H,
                                     name=f"h2o_{sc}_{ec}", tag="h2o")
                    for sl in range(3):
                        lo = sl * 512
                        hi = min(lo + 512, HID)
                        ps = mmp.tile([P, hi - lo], F32)
                        for k in range(5):
                            lhsT = (xgT_cur[:, k, ec * P:(ec + 1) * P]
                                    if k < 4 else
                                    ea_cur[:, ec * P:(ec + 1) * P])
                            nc.tensor.matmul(
                                ps[:], lhsT, W1t[:, k, lo:hi],
                                start=(k == 0), stop=(k == 4))
                        nc.scalar.activation(
                            h2ot[:, lo:hi], ps[:], RELU,
                            bias=0.0, scale=scEt[:, c:c + 1])
                        drip_T(1)
                    r0 = c * P
                    nc.sync.dma_start(h2_d.ap()[r0:r0 + P, :], h2ot[:])
                drip_T(16)
                xg_cur, ea_cur, xgT_cur = xg_next, ea_next, xgT_next

        # ------- Phases S+N: scatter-sum + per-node W2 + node MLP -------
        with ExitStack() as sctx:
            wpool2 = sctx.enter_context(tc.tile_pool(name="wN", bufs=1))
            W3xt = wpool2.tile([P, 4, INS], DT_X3)
            nc.sync.dma_start(
                W3xt[:], W3x_d.ap().rearrange("(ko ki) m -> ki ko m", ki=P))
            W3at = wpool2.tile([P, 10, INS], DT_A)
            nc.sync.dma_start(
                W3at[:], W3a_d.ap().rearrange("(ko ki) m -> ki ko m", ki=P))
            W4t = wpool2.tile([P, 5, FN], DT_H3)
            nc.sync.dma_start(
                W4t[:], W4_d.ap().rearrange("(ko ki) m -> ki ko m", ki=P))

            h2gp = sctx.enter_context(tc.tile_pool(name="h2g", bufs=3 * KB))
            Sp = sctx.enter_context(tc.tile_pool(name="Smat", bufs=3 * KB))
            aggTp = sctx.enter_context(tc.tile_pool(name="aggT", bufs=2))
            xsp = sctx.enter_context(tc.tile_pool(name="xs", bufs=2))
            h3p = sctx.enter_context(tc.tile_pool(name="h3T", bufs=2))
            ogp = sctx.enter_context(tc.tile_pool(name="og", bufs=4))
            smp = sctx.enter_context(
                tc.tile_pool(name="smp", bufs=6, space="PSUM"))
            mmp2 = sctx.enter_context(
                tc.tile_pool(name="mmN", bufs=2, space="PSUM"))

            # Rolling gather lookahead: block b's h2-row gathers (slow,
            # gpsimd SW-DGE) are issued two blocks ahead of its scatter
            # matmuls.  Pad slots carry an out-of-bounds id and are
            # silently skipped by the DMA (bounds_check); their S columns
            # are all-zero so stale SBUF data never contributes.
            pend_gs = {}

            def gather_S(b):
                lst = []
                for k in range(KB):
                    c = b * KB + k
                    h2g = h2gp.tile([P, HID], DT_H, name=f"h2g_{b}_{k}",
                                    tag="h2g")
                    St = Sp.tile([P, P], DT_H, name=f"S_{b}_{k}", tag="S")
                    nc.gpsimd.indirect_dma_start(
                        out=h2g[:], out_offset=None, in_=h2_d.ap()[:],
                        in_offset=bass.IndirectOffsetOnAxis(
                            ap=gidt[:, c:c + 1], axis=0),
                        bounds_check=EP - 1, oob_is_err=False)
                    nc.vector.tensor_tensor(
                        St[:], colbt[:, c:c + 1].to_broadcast([P, P]),
                        iotat[:], op=mybir.AluOpType.is_equal)
                    lst.append((h2g, St))
                pend_gs[b] = lst

            gather_S(0)
            gather_S(1)

            def load_xst(s):
                xst = xsp.tile([P, 4, 512], DT_X3, name=f"xst_{s}", tag="xst")
                nc.sync.dma_start(
                    xst[:],
                    xsT_d.ap().rearrange("(fo fi) n -> fi fo n", fi=P)
                    [:, :, s * 512:(s + 1) * 512])
                return xst

            def do_scatter(s):
                aggTt = aggTp.tile([P, 10, 512], DT_A)
                for bb in range(4):
                    b = s * 4 + bb
                    if b + 2 < NB:
                        gather_S(b + 2)
                    # scatter directly in transposed form:
                    #   aggT[f*128:(f+1)*128, node] += h2g[:, fslice]^T @ S
                    # 4 f-slices share one bank-sized PSUM tile (separate
                    # accumulation regions via per-slice start/stop).
                    psf = [smp.tile([P, min(4, 10 - 4 * g) * P], F32,
                                    name=f"ps_{b}_{g}", tag="psf")
                           for g in range(3)]
                    # NOTE: the PSUM start bit zeroes the whole 2KB bank
                    # (ZERO_REGION_SIZE), so emit start=True only on the
                    # first matmul into each bank tile; later regions
                    # auto-initialize via the pending-zero bytes.
                    for k, (h2g, St) in enumerate(pend_gs.pop(b)):
                        for f in range(10):
                            g = f // 4
                            fl = f % 4
                            nfg = min(4, 10 - 4 * g)
                            dst = psf[g][:, fl * P:(fl + 1) * P]
                            nc.tensor.matmul(
                                dst, h2g[:, f * P:(f + 1) * P], St[:],
                                start=(k == 0 and fl == 0),
                                stop=(k == KB - 1 and fl == nfg - 1),
                                skip_group_check=True)
                    for g in range(3):
                        nf = min(4, 10 - 4 * g)
                        nc.vector.tensor_copy(
                            aggTt[:, 4 * g:4 * g + nf,
                                  bb * P:(bb + 1) * P], psf[g][:])
                return aggTt

            aggT_cur = do_scatter(0)
            xst_cur = load_xst(0)
            for s in range(NSB):
                xst = xst_cur
                xst_cur = load_xst(s + 1) if s + 1 < NSB else None
                h3Tt = h3p.tile([P, 5, 512], DT_H3)
                for of in range(5):
                    ps = mmp2.tile([P, 512], F32)
                    for k in range(4):
                        nc.tensor.matmul(
                            ps[:], W3xt[:, k, of * P:(of + 1) * P],
                            xst[:, k, :], start=(k == 0), stop=False)
                    for f in range(10):
                        nc.tensor.matmul(
                            ps[:], W3at[:, f, of * P:(of + 1) * P],
                            aggT_cur[:, f, :], start=False, stop=(f == 9))
                    nc.scalar.activation(h3Tt[:, of, :], ps[:], RELU,
                                         bias=b3t[:, of:of + 1],
                                         scale=is3t[:, 0:1])
                # next superblock's scatter here: its matmuls and copies
                # hide the h3T drain latency before MM4 reads it.
                aggT_next = do_scatter(s + 1) if s + 1 < NSB else None
                # MM4 node-major: out[node, feat] = h3T slices @ W4 (moving)
                for nb in range(4):
                    ps = mmp2.tile([P, FN], F32)
                    if H3_F8:
                        nc.tensor.matmul(
                            ps[:], h3Tt[:, 0:2, nb * P:(nb + 1) * P],
                            W4t[:, 0:2, :], start=True, stop=False,
                            perf_mode=DR)
                        nc.tensor.matmul(
                            ps[:], h3Tt[:, 2:4, nb * P:(nb + 1) * P],
                            W4t[:, 2:4, :], start=False, stop=False,
                            perf_mode=DR)
                        nc.tensor.matmul(
                            ps[:], h3Tt[:, 4, nb * P:(nb + 1) * P],
                            W4t[:, 4, :], start=False, stop=True)
                    else:
                        for k in range(5):
                            nc.tensor.matmul(
                                ps[:], h3Tt[:, k, nb * P:(nb + 1) * P],
                                W4t[:, k, :], start=(k == 0), stop=(k == 4))
                    ogt = ogp.tile([P, FN], F32, name=f"og_{s}_{nb}",
                                   tag="og")
                    nc.scalar.activation(ogt[:], ps[:], COPY,
                                         bias=0.0, scale=is4t[:, 0:1])
                    r0 = s * 512 + nb * P
                    nc.sync.dma_start(out_d.ap()[r0:r0 + P, :], ogt[:])
                aggT_cur = aggT_next

    nc.compile()
    return nc


def _prepare(x, row, col, ea):
    """Host-side sharding: sort edges by destination, split nodes into 8
    block-aligned edge-balanced shards, build per-core arrays."""
    N = x.shape[0]
    E = ea.shape[0]
    order = np.argsort(col, kind="stable")
    scol = col[order]
    srow = row[order]
    NBLK = (N + P - 1) // P
    NTOT = NBLK * P

    bounds = [0]
    for p in range(1, NCORES):
        if E > 0:
            t = int(scol[min((p * E) // NCORES, E - 1)])
        else:
            t = (p * NTOT) // NCORES
        b = int(round(t / P)) * P
        b = max(b, bounds[-1] + P)
        b = min(b, NTOT - P * (NCORES - p))
        bounds.append(b)
    bounds.append(NTOT)
    for p in range(1, NCORES + 1):
        assert bounds[p] > bounds[p - 1], f"degenerate shard bounds {bounds}"

    e_split = np.searchsorted(scol, bounds)
    Ec = np.diff(e_split)
    EC = max(4, math.ceil(int(Ec.max()) / P))
    EC = ((EC + 3) // 4) * 4
    EP = EC * P
    nblk = [(bounds[p + 1] - bounds[p]) // P for p in range(NCORES)]
    NB = max(4, ((max(nblk) + 3) // 4) * 4)
    NBP = NB * P
    blkdeg = np.bincount(scol // P, minlength=NBLK)
    KB = max(1, math.ceil(int(blkdeg.max()) / P))

    cnt_full = np.bincount(col, minlength=N).astype(np.float32)
    inv_cnt = 1.0 / np.maximum(cnt_full, 1.0)

    xq = np.asarray(x, dtype=_npdt(EDGE_F8))          # replicated gather src
    xpadT = np.zeros((FN, NTOT + NBP), _npdt(X3_F8))
    xpadT[:, :N] = np.asarray(x, dtype=_npdt(X3_F8)).T

    cores = []
    for p in range(NCORES):
        s, e = int(e_split[p]), int(e_split[p + 1])
        n0 = bounds[p]
        ne = e - s
        tmp = np.zeros(EP, np.int32)
        tmp[:ne] = srow[s:e]
        rows_t = np.ascontiguousarray(tmp.reshape(EC, P).T)
        eaT = np.zeros((FE, EP), _npdt(EDGE_F8))
        eaT[:, :ne] = np.asarray(ea[order[s:e]], dtype=_npdt(EDGE_F8)).T
        # per-edge drain scale: 1/cnt(dest); padded slots scale to 0
        scE = np.zeros(EP, np.float32)
        scE[:ne] = inv_cnt[scol[s:e]]
        scE_t = np.ascontiguousarray(scE.reshape(EC, P).T)
        lcol = (scol[s:e] - n0).astype(np.int64)
        bstart = np.searchsorted(lcol, np.arange(NB + 1) * P)
        gid = np.full((NB, KB, P), 1 << 30, np.int32)
        # warmup window: the first 3*KB gather tiles come from fresh
        # (uninitialized) SBUF buffers; point their pad slots at row 0 so
        # skipped transfers never leave NaN bytes under the S=0 mask.
        gid.reshape(NB * KB, P)[:3 * KB + 2] = 0
        colb = np.full((NB, KB, P), -1.0, np.float32)
        for b in range(NB):
            sb, eb = int(bstart[b]), int(bstart[b + 1])
            cnt = eb - sb
            assert cnt <= KB * P
            gid[b].reshape(-1)[:cnt] = np.arange(sb, eb, dtype=np.int32)
            colb[b].reshape(-1)[:cnt] = (lcol[sb:eb] - b * P)
        gid_t = np.ascontiguousarray(gid.reshape(NB * KB, P).T)
        colb_t = np.ascontiguousarray(colb.reshape(NB * KB, P).T)
        xsT = np.ascontiguousarray(xpadT[:, n0:n0 + NBP])
        cores.append(dict(rows=rows_t, eaT=eaT, scE=scE_t, gid=gid_t,
                          colb=colb_t, xsT=xsT))
    return cores, bounds, EC, NB, KB, xq


def _run(inputs, trace=False):
    x = np.ascontiguousarray(np.asarray(inputs["x"], dtype=np.float32))
    ei = np.asarray(inputs["edge_index"])
    ea = np.ascontiguousarray(np.asarray(inputs["edge_attr"], dtype=np.float32))
    row = ei[0].astype(np.int64)
    col = ei[1].astype(np.int64)
    W1 = np.asarray(inputs["W1"], np.float32)
    W2 = np.asarray(inputs["W2"], np.float32)
    W3 = np.asarray(inputs["W3"], np.float32)
    W4 = np.asarray(inputs["W4"], np.float32)
    b1 = np.asarray(inputs["b1"], np.float32)
    b2 = np.asarray(inputs["b2"], np.float32)
    b3 = np.asarray(inputs["b3"], np.float32)
    b4 = np.asarray(inputs["b4"], np.float32)
    N = x.shape[0]
    # b1/b2/b4 are zero in this model (jnp.zeros in setup); the edge-major
    # drains and the mean/W2 commutation rely on it.  b3 stays general.
    assert not b1.any() and not b2.any() and not b4.any(), \
        "nonzero b1/b2/b4 unsupported"

    cores, bounds, EC, NB, KB, xq = _prepare(x, row, col, ea)

    key = (EC, NB, KB, N)
    if key not in _prog_cache:
        _prog_cache[key] = _build(EC, NB, KB, N)
    nc = _prog_cache[key]

    s1 = _pow2scale(W1) if EDGE_F8 else 1.0
    s3 = _pow2scale(W3) if (X3_F8 or AGG_F8) else 1.0
    s4 = _pow2scale(W4) if H3_F8 else 1.0
    W1q = np.ascontiguousarray((W1 * s1).astype(_npdt(EDGE_F8)))
    W3xq = np.ascontiguousarray((W3[:FN] * s3).astype(_npdt(X3_F8)))
    W23 = W2.astype(np.float64) @ W3[FN:].astype(np.float64)
    W3aq = np.ascontiguousarray((W23 * s3).astype(_npdt(AGG_F8)))
    W4q = np.ascontiguousarray((W4 * s4).astype(_npdt(H3_F8)))

    b1t = np.ascontiguousarray(b1.reshape(HID // P, P).T)
    b3t = np.ascontiguousarray(b3.reshape(INS // P, P).T)
    iota = np.ascontiguousarray(
        np.broadcast_to(np.arange(P, dtype=np.float32), (P, P)))
    idE = np.eye(P).astype(_npdt(EDGE_F8))
    ones = np.ones((P, 1), np.float32)

    in_maps = []
    for p in range(NCORES):
        c = cores[p]
        in_maps.append({
            "x": xq, "rows": c["rows"], "eaT": c["eaT"],
            "W1": W1q, "W3x": W3xq, "W3a": W3aq, "W4": W4q,
            "b1": b1t, "b3": b3t, "scE": c["scE"] / s1,
            "inv_s1": ones / s1,
            "inv_s3": ones / s3, "inv_s4": ones / s4,
            "gid": c["gid"], "colb": c["colb"],
            "xsT": c["xsT"], "iota": iota, "idE": idE,
        })

    res = run_bass_kernel_spmd(nc, in_maps, list(range(NCORES)), trace=trace)

    out = np.empty((N, FN), np.float32)
    for p in range(NCORES):
        n0, n1 = bounds[p], min(bounds[p + 1], N)
        if n1 > n0:
            out[n0:n1] = res.results[p]["out"][:n1 - n0]
    return out, res


def kernel(**inputs) -> np.ndarray:
    out, _ = _run(inputs, trace=False)
    return out


# revision 44
# speedup vs baseline: 1.0423x; 1.0423x over previous
"""Trainium2 Bass kernel for nn_NodeModel (GNN message passing).

Reference computation:
    h   = relu(concat(x[row], edge_attr) @ W1 + b1) @ W2 + b2     # edge MLP
    agg = scatter_mean(h, col, N)                                  # per-dest mean
    out = relu(concat(x, agg) @ W3 + b3) @ W4 + b4                 # node MLP

Distribution strategy (8 cores, no collectives needed):
  - Sort edges by destination node; split destination nodes into 8
    block-aligned, edge-balanced shards.  Each core owns one node shard and
    ALL edges targeting it, so per-node sums are complete locally.
  - x is replicated; each core gathers x[row] rows with indirect DMA.

Key design points (vs the fp32r baseline at ~2.0ms; now ~0.80ms):
  - All matmuls in bf16 (1 cycle/row on the PE, like fp32r, but half the
    SBUF/DMA footprint).  fp8+DoubleRow was measured and rejected: any
    single e4m3-quantized operand already costs 1.5-3e-2 relative error
    vs the 2e-2 tolerance (errors compose in quadrature).
  - scatter_mean commutes with the (linear) W2 matmul: only
    g = relu(cat @ W1)/cnt is computed per edge; W2 is applied per node.
    Further, g only feeds z @ W3, so W2 @ W3[FN:] is folded into ONE
    host-precomputed weight W23: the entire per-node W2 stage vanishes.
    Device FLOPs drop ~2.4x vs the reference dataflow.
  - MM1 emits edge-major (W1 moving, cat^T stationary), so the per-edge
    tensor g needs no exit transposes.  relu and the per-edge 1/cnt
    scale (a per-partition scale in edge-major layout) fold into the
    PSUM drain.  Relies on b1 == b2 == 0 (asserted; true for this model).
  - The scatter emits agg^T directly (aggT[f,n] += g[e,fslice]^T @ S[e,n]
    with one-hot S built by is_equal): no agg transposes.  PSUM start
    bits zero a whole 2KB bank, so the four f-regions sharing a bank get
    one start=True on first touch only.
  - MM4 streams W4 with h3^T stationary, emitting output node-major: no
    output transposes.  Phase 2 contains zero PE transposes (HAM-safe).
  - g staged to DRAM in bf16 and gathered per 128-node block by SW-DGE
    (schedule width KB = max chunks per block, exact).
"""

import math
import sys
from contextlib import ExitStack

sys.path.insert(0, "/opt/trn_rl_repo")

import ml_dtypes
import numpy as np

import concourse.bass as bass
import concourse.tile as tile
from concourse import bacc, mybir
from concourse.bass_utils import run_bass_kernel_spmd

NCORES = 8
P = 128
FN = 512    # node feature dim
FE = 128    # edge feature dim
HID = 1280  # edge-MLP hidden/output dim
INS = FN + FE           # 640  edge-MLP input
IN2 = FN + HID          # 1792 node-MLP input
F32 = mybir.dt.float32
BF16 = mybir.dt.bfloat16
F8 = mybir.dt.float8e4
I32 = mybir.dt.int32
RELU = mybir.ActivationFunctionType.Relu
COPY = mybir.ActivationFunctionType.Copy
DR = mybir.MatmulPerfMode.DoubleRow
MULT = mybir.AluOpType.mult

# ---- precision config (validated against the reference on CPU) ----
EDGE_F8 = False    # x-gather, edge_attr, W1, h1, W2
H2O_F8 = False     # h2 staging + scatter (False -> bf16)
X3_F8 = False      # x rows in node MLP + W3 x-part
AGG_F8 = False     # agg (aggT) + W3 agg-part
H3_F8 = False      # h3 + W4

NP_F8 = ml_dtypes.float8_e4m3
NP_BF16 = ml_dtypes.bfloat16


def _dt(flag):
    return F8 if flag else BF16


def _npdt(flag):
    return NP_F8 if flag else NP_BF16


def _pow2scale(w):
    m = float(np.abs(w).max())
    if m == 0.0:
        return 1.0
    return 2.0 ** math.floor(math.log2(224.0 / m)) / 2


_prog_cache = {}


def _build(EC, NB, KB, NX):
    """Build the SPMD program for one core.

    EC: edge chunks (128 edges each) per core, multiple of 4.
    NB: node blocks (128 nodes each) per core, multiple of 4.
    KB: max edge chunks per node block (scatter schedule width), even.
    NX: number of rows of the replicated x (gather source).
    """
    EP = EC * P
    NBP = NB * P
    SC = EC // 4   # superchunks of 512 edges
    NSB = NB // 4  # superblocks of 512 nodes

    DT_E = _dt(EDGE_F8)
    DT_H = _dt(H2O_F8)
    DT_X3 = _dt(X3_F8)
    DT_A = _dt(AGG_F8)
    DT_H3 = _dt(H3_F8)

    nc = bacc.Bacc("TRN2", target_bir_lowering=False, debug=False,
                   num_devices=NCORES)

    x_d = nc.dram_tensor("x", [NX, FN], DT_E, kind="ExternalInput")
    rows_d = nc.dram_tensor("rows", [P, EC], I32, kind="ExternalInput")
    eaT_d = nc.dram_tensor("eaT", [FE, EP], DT_E, kind="ExternalInput")
    W1_d = nc.dram_tensor("W1", [INS, HID], DT_E, kind="ExternalInput")
    W3x_d = nc.dram_tensor("W3x", [FN, INS], DT_X3, kind="ExternalInput")
    # W3a holds the host-precomputed W2 @ W3[FN:]: the scatter-mean output
    # g feeds W3 through W2, both linear, so the two weights fuse.
    W3a_d = nc.dram_tensor("W3a", [HID, INS], DT_A, kind="ExternalInput")
    W4_d = nc.dram_tensor("W4", [INS, FN], DT_H3, kind="ExternalInput")
    b1_d = nc.dram_tensor("b1", [P, HID // P], F32, kind="ExternalInput")
    b3_d = nc.dram_tensor("b3", [P, INS // P], F32, kind="ExternalInput")
    scE_d = nc.dram_tensor("scE", [P, EC], F32, kind="ExternalInput")
    gid_d = nc.dram_tensor("gid", [P, NB * KB], I32, kind="ExternalInput")
    colb_d = nc.dram_tensor("colb", [P, NB * KB], F32, kind="ExternalInput")
    xsT_d = nc.dram_tensor("xsT", [FN, NBP], DT_X3, kind="ExternalInput")
    iota_d = nc.dram_tensor("iota", [P, P], F32, kind="ExternalInput")
    idE_d = nc.dram_tensor("idE", [P, P], DT_E, kind="ExternalInput")
    out_d = nc.dram_tensor("out", [NBP, FN], F32, kind="ExternalOutput")
    h2_d = nc.dram_tensor("h2buf", [EP, HID], DT_H)  # internal staging

    # weight scales folded into drains (host passes pre-scaled weights)
    inv_s1_d = nc.dram_tensor("inv_s1", [P, 1], F32, kind="ExternalInput")
    inv_s3_d = nc.dram_tensor("inv_s3", [P, 1], F32, kind="ExternalInput")
    inv_s4_d = nc.dram_tensor("inv_s4", [P, 1], F32, kind="ExternalInput")

    with tile.TileContext(nc) as tc, ExitStack() as ctx:
        cpool = ctx.enter_context(tc.tile_pool(name="const", bufs=1))

        idEt = cpool.tile([P, P], DT_E)
        nc.sync.dma_start(idEt[:], idE_d.ap()[:])
        iotat = cpool.tile([P, P], F32)
        nc.sync.dma_start(iotat[:], iota_d.ap()[:])
        b1t = cpool.tile([P, HID // P], F32)
        nc.sync.dma_start(b1t[:], b1_d.ap()[:])
        b3t = cpool.tile([P, INS // P], F32)
        nc.sync.dma_start(b3t[:], b3_d.ap()[:])
        scEt = cpool.tile([P, EC], F32)
        nc.sync.dma_start(scEt[:], scE_d.ap()[:])
        rowst = cpool.tile([P, EC], I32)
        nc.sync.dma_start(rowst[:], rows_d.ap()[:])
        gidt = cpool.tile([P, NB * KB], I32)
        nc.sync.dma_start(gidt[:], gid_d.ap()[:])
        colbt = cpool.tile([P, NB * KB], F32)
        nc.sync.dma_start(colbt[:], colb_d.ap()[:])
        is1t = cpool.tile([P, 1], F32)
        nc.sync.dma_start(is1t[:], inv_s1_d.ap()[:])
        is3t = cpool.tile([P, 1], F32)
        nc.sync.dma_start(is3t[:], inv_s3_d.ap()[:])
        is4t = cpool.tile([P, 1], F32)
        nc.sync.dma_start(is4t[:], inv_s4_d.ap()[:])

        # Phase-2 weights + first x-shard tile: loaded up-front so their
        # DMAs don't queue behind all of phase 1's h2-staging writes.
        # ---------------- Phase E: edge half-MLP ----------------
        # Stages g_e = relu(cat(x[row], ea) @ W1) / cnt[col(e)] per edge.
        # scatter_mean commutes with the (linear) W2 matmul + b2 (b2==0),
        # so W2 is applied per *node* in phase 2: 2.56x less W2 work.
        with ExitStack() as ectx:
            wpool = ectx.enter_context(tc.tile_pool(name="wE", bufs=1))
            W1t = wpool.tile([P, 5, HID], DT_E)
            W1r = W1_d.ap().rearrange("(ko ki) m -> ki ko m", ki=P)
            for k in range(5):
                nc.sync.dma_start(W1t[:, k, :], W1r[:, k, :])

            ptp = ectx.enter_context(
                tc.tile_pool(name="ptp", bufs=2, space="PSUM"))
            xgp = ectx.enter_context(tc.tile_pool(name="xg", bufs=2))
            xgTp = ectx.enter_context(tc.tile_pool(name="xgT", bufs=2))
            eap = ectx.enter_context(tc.tile_pool(name="ea", bufs=2))
            h2op = ectx.enter_context(tc.tile_pool(name="h2o", bufs=4))
            mmp = ectx.enter_context(
                tc.tile_pool(name="mmE", bufs=4, space="PSUM"))

            def issue_gather(sc):
                xgt = xgp.tile([P, 4, FN], DT_E)
                for k in range(4):
                    nc.gpsimd.indirect_dma_start(
                        out=xgt[:, k, :], out_offset=None, in_=x_d.ap()[:],
                        in_offset=bass.IndirectOffsetOnAxis(
                            ap=rowst[:, sc * 4 + k:sc * 4 + k + 1], axis=0))
                eat = eap.tile([P, 512], DT_E)
                nc.sync.dma_start(
                    eat[:], eaT_d.ap()[:, sc * 512:(sc + 1) * 512])
                return xgt, eat

            def entry_T(xgt, xgTt, f, k):
                pt = ptp.tile([P, P], DT_E)
                nc.tensor.transpose(
                    pt[:], xgt[:, k, f * P:(f + 1) * P], idEt[:])
                nc.vector.tensor_copy(xgTt[:, f, k * P:(k + 1) * P], pt[:])

            # prologue: superchunk 0 input + its entry transposes
            xg_cur, ea_cur = issue_gather(0)
            xgT_cur = xgTp.tile([P, 4, 512], DT_E)
            for f in range(4):
                for k in range(4):
                    entry_T(xg_cur, xgT_cur, f, k)

            for sc in range(SC):
                if sc + 1 < SC:
                    xg_next, ea_next = issue_gather(sc + 1)
                    xgT_next = xgTp.tile([P, 4, 512], DT_E)
                    tq = [(f, k) for f in range(4) for k in range(4)]
                else:
                    xg_next = ea_next = xgT_next = None
                    tq = []

                def drip_T(n):
                    for _ in range(n):
                        if tq:
                            f, k = tq.pop(0)
                            entry_T(xg_next, xgT_next, f, k)

                # MM1 edge-major: per 128-edge chunk, W1 moving,
                # cat^T slices stationary.  Drain: relu then scale by
                # (1/cnt)/s1 per edge (b1==0; relu commutes with the
                # positive scale).
                for ec in range(4):
                    c = sc * 4 + ec
                    h2ot = h2op.tile([P, HID], DT_H,
                                     name=f"h2o_{sc}_{ec}", tag="h2o")
                    for sl in range(3):
                        lo = sl * 512
                        hi = min(lo + 512, HID)
                        ps = mmp.tile([P, hi - lo], F32)
                        for k in range(5):
                            lhsT = (xgT_cur[:, k, ec * P:(ec + 1) * P]
                                    if k < 4 else
                                    ea_cur[:, ec * P:(ec + 1) * P])
                            nc.tensor.matmul(
                                ps[:], lhsT, W1t[:, k, lo:hi],
                                start=(k == 0), stop=(k == 4))
                        if sl == 1:
                            nc.vector.tensor_scalar(
                                h2ot[:, lo:hi], ps[:], scEt[:, c:c + 1],
                                0.0, op0=MULT,
                                op1=mybir.AluOpType.max)
                        else:
                            nc.scalar.activation(
                                h2ot[:, lo:hi], ps[:], RELU,
                                bias=0.0, scale=scEt[:, c:c + 1])
                        drip_T(1)
                    r0 = c * P
                    nc.sync.dma_start(h2_d.ap()[r0:r0 + P, :], h2ot[:])
                drip_T(16)
                xg_cur, ea_cur, xgT_cur = xg_next, ea_next, xgT_next

        # ------- Phases S+N: scatter-sum + per-node W2 + node MLP -------
        with ExitStack() as sctx:
            wpool2 = sctx.enter_context(tc.tile_pool(name="wN", bufs=1))
            W3xt = wpool2.tile([P, 4, INS], DT_X3)
            nc.sync.dma_start(
                W3xt[:], W3x_d.ap().rearrange("(ko ki) m -> ki ko m", ki=P))
            W3at = wpool2.tile([P, 10, INS], DT_A)
            nc.sync.dma_start(
                W3at[:], W3a_d.ap().rearrange("(ko ki) m -> ki ko m", ki=P))
            W4t = wpool2.tile([P, 5, FN], DT_H3)
            nc.sync.dma_start(
                W4t[:], W4_d.ap().rearrange("(ko ki) m -> ki ko m", ki=P))

            h2gp = sctx.enter_context(tc.tile_pool(name="h2g", bufs=3 * KB))
            Sp = sctx.enter_context(tc.tile_pool(name="Smat", bufs=3 * KB))
            aggTp = sctx.enter_context(tc.tile_pool(name="aggT", bufs=2))
            xsp = sctx.enter_context(tc.tile_pool(name="xs", bufs=2))
            h3p = sctx.enter_context(tc.tile_pool(name="h3T", bufs=2))
            ogp = sctx.enter_context(tc.tile_pool(name="og", bufs=4))
            smp = sctx.enter_context(
                tc.tile_pool(name="smp", bufs=6, space="PSUM"))
            mmp2 = sctx.enter_context(
                tc.tile_pool(name="mmN", bufs=2, space="PSUM"))

            # Rolling gather lookahead: block b's h2-row gathers (slow,
            # gpsimd SW-DGE) are issued two blocks ahead of its scatter
            # matmuls.  Pad slots carry an out-of-bounds id and are
            # silently skipped by the DMA (bounds_check); their S columns
            # are all-zero so stale SBUF data never contributes.
            pend_gs = {}

            def gather_S(b):
                lst = []
                for k in range(KB):
                    c = b * KB + k
                    h2g = h2gp.tile([P, HID], DT_H, name=f"h2g_{b}_{k}",
                                    tag="h2g")
                    St = Sp.tile([P, P], DT_H, name=f"S_{b}_{k}", tag="S")
                    nc.gpsimd.indirect_dma_start(
                        out=h2g[:], out_offset=None, in_=h2_d.ap()[:],
                        in_offset=bass.IndirectOffsetOnAxis(
                            ap=gidt[:, c:c + 1], axis=0),
                        bounds_check=EP - 1, oob_is_err=False)
                    nc.vector.tensor_tensor(
                        St[:], colbt[:, c:c + 1].to_broadcast([P, P]),
                        iotat[:], op=mybir.AluOpType.is_equal)
                    lst.append((h2g, St))
                pend_gs[b] = lst

            gather_S(0)
            gather_S(1)

            def load_xst(s):
                xst = xsp.tile([P, 4, 512], DT_X3, name=f"xst_{s}", tag="xst")
                nc.sync.dma_start(
                    xst[:],
                    xsT_d.ap().rearrange("(fo fi) n -> fi fo n", fi=P)
                    [:, :, s * 512:(s + 1) * 512])
                return xst

            def do_scatter(s):
                aggTt = aggTp.tile([P, 10, 512], DT_A)
                for bb in range(4):
                    b = s * 4 + bb
                    if b + 2 < NB:
                        gather_S(b + 2)
                    # scatter directly in transposed form:
                    #   aggT[f*128:(f+1)*128, node] += h2g[:, fslice]^T @ S
                    # 4 f-slices share one bank-sized PSUM tile (separate
                    # accumulation regions via per-slice start/stop).
                    psf = [smp.tile([P, min(4, 10 - 4 * g) * P], F32,
                                    name=f"ps_{b}_{g}", tag="psf")
                           for g in range(3)]
                    # NOTE: the PSUM start bit zeroes the whole 2KB bank
                    # (ZERO_REGION_SIZE), so emit start=True only on the
                    # first matmul into each bank tile; later regions
                    # auto-initialize via the pending-zero bytes.
                    for k, (h2g, St) in enumerate(pend_gs.pop(b)):
                        for f in range(10):
                            g = f // 4
                            fl = f % 4
                            nfg = min(4, 10 - 4 * g)
                            dst = psf[g][:, fl * P:(fl + 1) * P]
                            nc.tensor.matmul(
                                dst, h2g[:, f * P:(f + 1) * P], St[:],
                                start=(k == 0 and fl == 0),
                                stop=(k == KB - 1 and fl == nfg - 1),
                                skip_group_check=True)
                    for g in range(3):
                        nf = min(4, 10 - 4 * g)
                        nc.vector.tensor_copy(
                            aggTt[:, 4 * g:4 * g + nf,
                                  bb * P:(bb + 1) * P], psf[g][:])
                return aggTt

            aggT_cur = do_scatter(0)
            xst_cur = load_xst(0)
            for s in range(NSB):
                xst = xst_cur
                xst_cur = load_xst(s + 1) if s + 1 < NSB else None
                h3Tt = h3p.tile([P, 5, 512], DT_H3)
                for of in range(5):
                    ps = mmp2.tile([P, 512], F32)
                    for k in range(4):
                        nc.tensor.matmul(
                            ps[:], W3xt[:, k, of * P:(of + 1) * P],
                            xst[:, k, :], start=(k == 0), stop=False)
                    for f in range(10):
                        nc.tensor.matmul(
                            ps[:], W3at[:, f, of * P:(of + 1) * P],
                            aggT_cur[:, f, :], start=False, stop=(f == 9))
                    nc.scalar.activation(h3Tt[:, of, :], ps[:], RELU,
                                         bias=b3t[:, of:of + 1],
                                         scale=is3t[:, 0:1])
                # next superblock's scatter here: its matmuls and copies
                # hide the h3T drain latency before MM4 reads it.
                aggT_next = do_scatter(s + 1) if s + 1 < NSB else None
                # MM4 node-major: out[node, feat] = h3T slices @ W4 (moving)
                for nb in range(4):
                    ps = mmp2.tile([P, FN], F32)
                    if H3_F8:
                        nc.tensor.matmul(
                            ps[:], h3Tt[:, 0:2, nb * P:(nb + 1) * P],
                            W4t[:, 0:2, :], start=True, stop=False,
                            perf_mode=DR)
                        nc.tensor.matmul(
                            ps[:], h3Tt[:, 2:4, nb * P:(nb + 1) * P],
                            W4t[:, 2:4, :], start=False, stop=False,
                            perf_mode=DR)
                        nc.tensor.matmul(
                            ps[:], h3Tt[:, 4, nb * P:(nb + 1) * P],
                            W4t[:, 4, :], start=False, stop=True)
                    else:
                        for k in range(5):
                            nc.tensor.matmul(
                                ps[:], h3Tt[:, k, nb * P:(nb + 1) * P],
                                W4t[:, k, :], start=(k == 0), stop=(k == 4))
                    ogt = ogp.tile([P, FN], F32, name=f"og_{s}_{nb}",
                                   tag="og")
                    nc.scalar.activation(ogt[:], ps[:], COPY,
                                         bias=0.0, scale=is4t[:, 0:1])
                    r0 = s * 512 + nb * P
                    nc.sync.dma_start(out_d.ap()[r0:r0 + P, :], ogt[:])
                aggT_cur = aggT_next

    nc.compile()
    return nc


def _prepare(x, row, col, ea):
    """Host-side sharding: sort edges by destination, split nodes into 8
    block-aligned edge-balanced shards, build per-core arrays."""
    N = x.shape[0]
    E = ea.shape[0]
    order = np.argsort(col, kind="stable")
    scol = col[order]
    srow = row[order]
    NBLK = (N + P - 1) // P
    NTOT = NBLK * P

    bounds = [0]
    for p in range(1, NCORES):
        if E > 0:
            t = int(scol[min((p * E) // NCORES, E - 1)])
        else:
            t = (p * NTOT) // NCORES
        b = int(round(t / P)) * P
        b = max(b, bounds[-1] + P)
        b = min(b, NTOT - P * (NCORES - p))
        bounds.append(b)
    bounds.append(NTOT)
    for p in range(1, NCORES + 1):
        assert bounds[p] > bounds[p - 1], f"degenerate shard bounds {bounds}"

    e_split = np.searchsorted(scol, bounds)
    Ec = np.diff(e_split)
    EC = max(4, math.ceil(int(Ec.max()) / P))
    EC = ((EC + 3) // 4) * 4
    EP = EC * P
    nblk = [(bounds[p + 1] - bounds[p]) // P for p in range(NCORES)]
    NB = max(4, ((max(nblk) + 3) // 4) * 4)
    NBP = NB * P
    blkdeg = np.bincount(scol // P, minlength=NBLK)
    KB = max(1, math.ceil(int(blkdeg.max()) / P))

    cnt_full = np.bincount(col, minlength=N).astype(np.float32)
    inv_cnt = 1.0 / np.maximum(cnt_full, 1.0)

    xq = np.asarray(x, dtype=_npdt(EDGE_F8))          # replicated gather src
    xpadT = np.zeros((FN, NTOT + NBP), _npdt(X3_F8))
    xpadT[:, :N] = np.asarray(x, dtype=_npdt(X3_F8)).T

    cores = []
    for p in range(NCORES):
        s, e = int(e_split[p]), int(e_split[p + 1])
        n0 = bounds[p]
        ne = e - s
        tmp = np.zeros(EP, np.int32)
        tmp[:ne] = srow[s:e]
        rows_t = np.ascontiguousarray(tmp.reshape(EC, P).T)
        eaT = np.zeros((FE, EP), _npdt(EDGE_F8))
        eaT[:, :ne] = np.asarray(ea[order[s:e]], dtype=_npdt(EDGE_F8)).T
        # per-edge drain scale: 1/cnt(dest); padded slots scale to 0
        scE = np.zeros(EP, np.float32)
        scE[:ne] = inv_cnt[scol[s:e]]
        scE_t = np.ascontiguousarray(scE.reshape(EC, P).T)
        lcol = (scol[s:e] - n0).astype(np.int64)
        bstart = np.searchsorted(lcol, np.arange(NB + 1) * P)
        gid = np.full((NB, KB, P), 1 << 30, np.int32)
        # warmup window: the first 3*KB gather tiles come from fresh
        # (uninitialized) SBUF buffers; point their pad slots at row 0 so
        # skipped transfers never leave NaN bytes under the S=0 mask.
        gid.reshape(NB * KB, P)[:3 * KB + 2] = 0
        colb = np.full((NB, KB, P), -1.0, np.float32)
        for b in range(NB):
            sb, eb = int(bstart[b]), int(bstart[b + 1])
            cnt = eb - sb
            assert cnt <= KB * P
            gid[b].reshape(-1)[:cnt] = np.arange(sb, eb, dtype=np.int32)
            colb[b].reshape(-1)[:cnt] = (lcol[sb:eb] - b * P)
        gid_t = np.ascontiguousarray(gid.reshape(NB * KB, P).T)
        colb_t = np.ascontiguousarray(colb.reshape(NB * KB, P).T)
        xsT = np.ascontiguousarray(xpadT[:, n0:n0 + NBP])
        cores.append(dict(rows=rows_t, eaT=eaT, scE=scE_t, gid=gid_t,
                          colb=colb_t, xsT=xsT))
    return cores, bounds, EC, NB, KB, xq


def _run(inputs, trace=False):
    x = np.ascontiguousarray(np.asarray(inputs["x"], dtype=np.float32))
    ei = np.asarray(inputs["edge_index"])
    ea = np.ascontiguousarray(np.asarray(inputs["edge_attr"], dtype=np.float32))
    row = ei[0].astype(np.int64)
    col = ei[1].astype(np.int64)
    W1 = np.asarray(inputs["W1"], np.float32)
    W2 = np.asarray(inputs["W2"], np.float32)
    W3 = np.asarray(inputs["W3"], np.float32)
    W4 = np.asarray(inputs["W4"], np.float32)
    b1 = np.asarray(inputs["b1"], np.float32)
    b2 = np.asarray(inputs["b2"], np.float32)
    b3 = np.asarray(inputs["b3"], np.float32)
    b4 = np.asarray(inputs["b4"], np.float32)
    N = x.shape[0]
    # b1/b2/b4 are zero in this model (jnp.zeros in setup); the edge-major
    # drains and the mean/W2 commutation rely on it.  b3 stays general.
    assert not b1.any() and not b2.any() and not b4.any(), \
        "nonzero b1/b2/b4 unsupported"

    cores, bounds, EC, NB, KB, xq = _prepare(x, row, col, ea)

    key = (EC, NB, KB, N)
    if key not in _prog_cache:
        _prog_cache[key] = _build(EC, NB, KB, N)
    nc = _prog_cache[key]

    s1 = _pow2scale(W1) if EDGE_F8 else 1.0
    s3 = _pow2scale(W3) if (X3_F8 or AGG_F8) else 1.0
    s4 = _pow2scale(W4) if H3_F8 else 1.0
    W1q = np.ascontiguousarray((W1 * s1).astype(_npdt(EDGE_F8)))
    W3xq = np.ascontiguousarray((W3[:FN] * s3).astype(_npdt(X3_F8)))
    W23 = W2.astype(np.float64) @ W3[FN:].astype(np.float64)
    W3aq = np.ascontiguousarray((W23 * s3).astype(_npdt(AGG_F8)))
    W4q = np.ascontiguousarray((W4 * s4).astype(_npdt(H3_F8)))

    b1t = np.ascontiguousarray(b1.reshape(HID // P, P).T)
    b3t = np.ascontiguousarray(b3.reshape(INS // P, P).T)
    iota = np.ascontiguousarray(
        np.broadcast_to(np.arange(P, dtype=np.float32), (P, P)))
    idE = np.eye(P).astype(_npdt(EDGE_F8))
    ones = np.ones((P, 1), np.float32)

    in_maps = []
    for p in range(NCORES):
        c = cores[p]
        in_maps.append({
            "x": xq, "rows": c["rows"], "eaT": c["eaT"],
            "W1": W1q, "W3x": W3xq, "W3a": W3aq, "W4": W4q,
            "b1": b1t, "b3": b3t, "scE": c["scE"] / s1,
            "inv_s1": ones / s1,
            "inv_s3": ones / s3, "inv_s4": ones / s4,
            "gid": c["gid"], "colb": c["colb"],
            "xsT": c["xsT"], "iota": iota, "idE": idE,
        })

    res = run_bass_kernel_spmd(nc, in_maps, list(range(NCORES)), trace=trace)

    out = np.empty((N, FN), np.float32)
    for p in range(NCORES):
        n0, n1 = bounds[p], min(bounds[p + 1], N)
        if n1 > n0:
            out[n0:n1] = res.results[p]["out"][:n1 - n0]
    return out, res


def kernel(**inputs) -> np.ndarray:
    out, _ = _run(inputs, trace=False)
    return out
